# revision 5
# baseline (speedup 1.0000x reference)
"""Expert-parallel MoE MLP (ExpertMLP) Bass kernel for 8 Trainium2 NeuronCores.

Problem: x[32,4096,256] @ w_fc[32,256,1024] -> gelu(erf) -> @ w_proj[32,1024,256].

Sharding: expert-parallel. Each of the 8 cores gets 4 experts (slices of the
leading axis of every tensor); no cross-core communication. Inside a core, per
expert e:

  1. x[e] ([4096,256], capacity-major) is cast to bf16 (DRAM->DRAM SWDGE cast
     on gpsimd) in 512-row slabs, then each slab is XBar DMA-transposed into
     SBUF as xT [d, c] so the d-contraction of MM1 lies on the partition axis.
     The PE never spends a cycle on transposes.
  2. MM1: hT[h_tile, c_chunk] += w_fc_tile.T @ xT_chunk - w_fc's natural
     [d, h] layout is the stationary operand, so it needs no transpose.
  3. GELU (exact erf form) runs on the ACT engine as the PSUM->SBUF eviction,
     writing bf16 hT tiles.
  4. MM2 uses hT slices as the *stationary* operand and w_proj's natural
     [h, d] layout as the moving operand: out[c_sub, d] += hT_slice.T @
     w_proj_tile. The result lands directly in [capacity, d] orientation, so
     no output transpose is needed.

Scheduling (what makes it fast):
  - Priority-ordered prologue: w_fc[e0] (scalar HWDGE queue) and the cast+
    transpose chain for e0's first slab are enqueued before everything else,
    so MM1 starts ~13us in instead of waiting ~48us for all casts to drain.
  - MM2 of chunk t runs after MM1 of chunk t+1 (one-chunk software pipeline),
    so the ACT-engine GELU of chunk t completes long before MM2 needs it and
    the PE never stalls on the activation.
  - Queue separation: weights go on the Act HWDGE queue, x-casts on the
    gpsimd SWDGE queue, transposes + output stores on the sync HWDGE queue.
    Transposes/casts for expert e+2 are issued inside expert e's loop so
    tile-pool aliasing never head-of-line-blocks the store stream.
"""

import numpy as np
from contextlib import ExitStack

import bass_rust as _br
import concourse.bass as bass
import concourse.tile as tile
from concourse import mybir
from concourse.bass_utils import run_bass_kernel_spmd

E, CAP, D, H = 32, 4096, 256, 1024
N_CORES = 8
E_PER = E // N_CORES  # 4 experts per core
P = 128
F32 = mybir.dt.float32
F32R = mybir.dt.float32r
BF16 = mybir.dt.bfloat16

KD = D // P        # 2 k-tiles in MM1's contraction
KH = H // P        # 8 k-tiles in MM2's contraction
NC_CHUNK = 512     # capacity chunk processed per MM1/MM2 round == slab size
N_CHUNKS = CAP // NC_CHUNK
H_TILES = H // P
HPACK = 2          # h_tiles packed per PSUM tile / GELU call
NS = NC_CHUNK // P
T_CHUNKS = E_PER * N_CHUNKS  # 32 global (expert, chunk) rounds


def _fix_waits(nc):
    """walrus here accepts only one sync wait per instruction; hoist excess
    waits onto standalone EventSemaphore instructions inserted before the
    offender (same engine => same sequencer order)."""
    for fn in nc.m.functions:
        for bb in fn.blocks:
            new = []
            changed = False
            for inst in bb.instructions:
                si = inst.sync_info
                if si is not None and len(si.on_wait) > 1:
                    waits = list(si.on_wait)
                    for w in waits[:-1]:
                        ev = mybir.InstEventSemaphore(
                            name=nc.get_next_instruction_name()
                        )
                        ev.engine = inst.engine
                        ev.sync_info = _br.SyncInfo(on_wait=[w], on_update=[])
                        nc.register_instruction(ev)
                        new.append(ev)
                    inst.sync_info = _br.SyncInfo(
                        on_wait=waits[-1:], on_update=list(si.on_update)
                    )
                    changed = True
                new.append(inst)
            if changed:
                bb.instructions = new


def _build():
    nc = bass.Bass(trn_type="TRN2", target_bir_lowering=False, debug=False)
    x = nc.dram_tensor("x", [E_PER, CAP, D], F32, kind="ExternalInput").ap()
    w_fc = nc.dram_tensor("w_fc", [E_PER, D, H], F32, kind="ExternalInput").ap()
    w_proj = nc.dram_tensor("w_proj", [E_PER, H, D], F32, kind="ExternalInput").ap()
    out = nc.dram_tensor("out", [E_PER, CAP, D], F32, kind="ExternalOutput").ap()

    with tile.TileContext(nc) as tc, ExitStack() as ctx:
        # 2 experts' worth of xT slabs in flight; ring aliasing gates the
        # transposes of expert e+2 on MM1 of expert e having consumed the slab.
        xtp = ctx.enter_context(tc.tile_pool(name="xtp", bufs=2 * N_CHUNKS * KD))
        xsp = ctx.enter_context(tc.tile_pool(name="xsp", bufs=4))
        wload = ctx.enter_context(tc.tile_pool(name="wload", bufs=2))
        wfc_p = ctx.enter_context(tc.tile_pool(name="wfc", bufs=2))
        wproj_p = ctx.enter_context(tc.tile_pool(name="wproj", bufs=2))
        ht_p = ctx.enter_context(tc.tile_pool(name="ht", bufs=8))
        out_p = ctx.enter_context(tc.tile_pool(name="outp", bufs=3))
        ps_h = ctx.enter_context(tc.tile_pool(name="ps_h", bufs=2, space="PSUM"))
        ps_o = ctx.enter_context(tc.tile_pool(name="ps_o", bufs=4, space="PSUM"))

        def load_weights(e):
            # raw f32 loads on the Act HWDGE queue (kept clear of the cast
            # and transpose streams), cast to bf16 on the idle DVE.
            wfc_raw = wload.tile([P, KD, H], F32, tag="wl")
            nc.scalar.dma_start(wfc_raw[:], w_fc[e].rearrange("(k p) h -> p k h", p=P))
            wfc = wfc_p.tile([P, KD, H], BF16, tag="wfc")
            nc.vector.tensor_copy(wfc[:], wfc_raw[:])
            wproj_raw = wload.tile([P, KH, D], F32, tag="wl")
            nc.scalar.dma_start(
                wproj_raw[:], w_proj[e].rearrange("(k p) d -> p k d", p=P)
            )
            wproj = wproj_p.tile([P, KH, D], BF16, tag="wproj")
            nc.vector.tensor_copy(wproj[:], wproj_raw[:])
            return wfc, wproj

        xts = [[[None] * N_CHUNKS for _ in range(KD)] for _ in range(E_PER)]

        def stage_slab(e, s):
            """cast x[e] slab s straight into SBUF as bf16 (gpsimd SWDGE cast,
            capacity-major) and XBar-transpose it SBUF->SBUF in 128x128 blocks
            (sync HWDGE). No DRAM round-trip: x is read from HBM exactly once."""
            rs = slice(s * NC_CHUNK, (s + 1) * NC_CHUNK)
            xs = xsp.tile([P, NS, D], BF16, tag="xs", name=f"xs{e}_{s}")
            nc.gpsimd.dma_start(
                xs[:], x[e][rs].rearrange("(b p) d -> p b d", p=P)
            )
            xt_k = []
            for k in range(KD):
                xt = xtp.tile([P, NC_CHUNK], BF16, tag="xt", name=f"xt{e}_{k}_{s}")
                xt_k.append(xt)
                xts[e][k][s] = xt
            for b in range(NS):
                for k in range(KD):
                    nc.sync.dma_start_transpose(
                        xt_k[k][:, b * P:(b + 1) * P],
                        xs[:, b, k * P:(k + 1) * P],
                    )

        # ---- prologue: e0's weights first, then per-slab cast->transpose
        # chains for experts 0 and 1 in consumption order. Experts 2 and 3
        # are staged inside the main loop (one slab per chunk round).
        w = [None] * E_PER
        w[0] = load_weights(0)
        for e in range(min(2, E_PER)):
            for s in range(N_CHUNKS):
                stage_slab(e, s)

        # pending MM2 work: (e, nci, ht_tiles) of the previous chunk round
        pend = None

        def run_mm2(p_e, p_nci, p_ht, last):
            wproj_t = w[p_e][1]
            psos = [
                ps_o.tile([P, 2 * D], F32, tag="pso",
                          name=f"pso{p_e}_{p_nci}_{i}")
                for i in range(NS)
            ]
            ob = out_p.tile([P, NS, D], F32, tag="ob")
            order = (
                [(s, k) for s in range(NS) for k in range(KH)]
                if last else
                [(s, k) for k in range(KH) for s in range(NS)]
            )
            for s, k in order:
                nc.tensor.matmul(
                    psos[s][:, :D],
                    p_ht[k // HPACK][:, k % HPACK, s * P:(s + 1) * P],
                    wproj_t[:, k, :],
                    start=(k == 0),
                    stop=(k == KH - 1),
                )
                if last and k == KH - 1:
                    # final round: per-subtile eviction+store so the output
                    # tail overlaps the last matmuls
                    nc.vector.tensor_copy(ob[:, s, :], psos[s][:, :D])
                    nc.sync.dma_start(
                        out[p_e, p_nci * NC_CHUNK + s * P:
                            p_nci * NC_CHUNK + (s + 1) * P, :],
                        ob[:, s, :],
                    )
            if not last:
                for s, pso in enumerate(psos):
                    nc.vector.tensor_copy(ob[:, s, :], pso[:, :D])
                csl = slice(p_nci * NC_CHUNK, (p_nci + 1) * NC_CHUNK)
                nc.sync.dma_start(
                    out[p_e, csl, :].rearrange("(s p) d -> p s d", p=P), ob[:]
                )

        for t in range(T_CHUNKS + 1):
            if t < T_CHUNKS:
                e, nci = divmod(t, N_CHUNKS)
                if nci == 0 and e + 1 < E_PER:
                    w[e + 1] = load_weights(e + 1)
                wfc_t = w[e][0]
                # ---- MM1 -> GELU for chunk t ----
                # MM1 accumulates HPACK h_tiles into one 2-bank PSUM tile so
                # GELU evicts in wide ACTIVATE calls; hT is written bf16 so
                # MM2's per-matmul weight loads run at 2-byte FWL speed.
                ht_tiles = []
                for hp in range(H_TILES // HPACK):
                    psh = ps_h.tile([P, HPACK, NC_CHUNK], F32, tag="psh")
                    for j in range(HPACK):
                        hi = hp * HPACK + j
                        for k in range(KD):
                            nc.tensor.matmul(
                                psh[:, j, :],
                                wfc_t[:, k, hi * P:(hi + 1) * P],
                                xts[e][k][nci][:],
                                start=(k == 0),
                                stop=(k == KD - 1),
                            )
                    ht = ht_p.tile([P, HPACK, NC_CHUNK], BF16, tag="ht")
                    nc.scalar.activation(
                        ht[:], psh[:], mybir.ActivationFunctionType.Gelu
                    )
                    ht_tiles.append(ht)
                # stage expert e+2's slab for this chunk position (keeps the
                # sync-queue transpose stream interleaved with output stores)
                if e + 2 < E_PER:
                    stage_slab(e + 2, nci)
            # ---- MM2 for the previous chunk round (one-chunk delay: its
            # GELUs completed during this round's MM1, so the PE never
            # waits on the ACT engine) ----
            if pend is not None:
                p_e, p_nci, p_ht = pend
                run_mm2(p_e, p_nci, p_ht, last=(t == T_CHUNKS))
            pend = (e, nci, ht_tiles) if t < T_CHUNKS else None

    _fix_waits(nc)
    return nc


_CACHE = {}


def _get_nc():
    if "nc" not in _CACHE:
        _CACHE["nc"] = _build()
    return _CACHE["nc"]


def kernel(x, w_fc, w_proj, trace=False):
    assert x.shape == (E, CAP, D) and w_fc.shape == (E, D, H)
    assert w_proj.shape == (E, H, D)
    nc = _get_nc()
    x = np.ascontiguousarray(x, dtype=np.float32)
    w_fc = np.ascontiguousarray(w_fc, dtype=np.float32)
    w_proj = np.ascontiguousarray(w_proj, dtype=np.float32)
    in_maps = [
        {
            "x": x[i * E_PER:(i + 1) * E_PER],
            "w_fc": w_fc[i * E_PER:(i + 1) * E_PER],
            "w_proj": w_proj[i * E_PER:(i + 1) * E_PER],
        }
        for i in range(N_CORES)
    ]
    res = run_bass_kernel_spmd(nc, in_maps, list(range(N_CORES)), trace=trace)
    out = np.concatenate([r["out"] for r in res.results], axis=0)
    if trace:
        kernel.last_results = res
    return out


# revision 9
# speedup vs baseline: 2.0285x; 2.0285x over previous
"""Expert-parallel MoE MLP (ExpertMLP) Bass kernel for 8 Trainium2 NeuronCores.

Problem: x[32,4096,256] @ w_fc[32,256,1024] -> gelu(erf) -> @ w_proj[32,1024,256].

Sharding: expert-parallel. Each of the 8 cores gets 4 experts (slices of the
leading axis of every tensor); no cross-core communication. Inside a core, per
expert e:

  1. x[e] ([4096,256], capacity-major) is cast to bf16 (DRAM->DRAM SWDGE cast
     on gpsimd) in 512-row slabs, then each slab is XBar DMA-transposed into
     SBUF as xT [d, c] so the d-contraction of MM1 lies on the partition axis.
     The PE never spends a cycle on transposes.
  2. MM1: hT[h_tile, c_chunk] += w_fc_tile.T @ xT_chunk - w_fc's natural
     [d, h] layout is the stationary operand, so it needs no transpose.
  3. GELU (exact erf form) runs on the ACT engine as the PSUM->SBUF eviction,
     writing bf16 hT tiles.
  4. MM2 uses hT slices as the *stationary* operand and w_proj's natural
     [h, d] layout as the moving operand: out[c_sub, d] += hT_slice.T @
     w_proj_tile. The result lands directly in [capacity, d] orientation, so
     no output transpose is needed.

Scheduling (what makes it fast):
  - Priority-ordered prologue: w_fc[e0] (scalar HWDGE queue) and the cast+
    transpose chain for e0's first slab are enqueued before everything else,
    so MM1 starts ~13us in instead of waiting ~48us for all casts to drain.
  - MM2 of chunk t runs after MM1 of chunk t+1 (one-chunk software pipeline),
    so the ACT-engine GELU of chunk t completes long before MM2 needs it and
    the PE never stalls on the activation.
  - Queue separation: weights go on the Act HWDGE queue, x-casts on the
    gpsimd SWDGE queue, transposes + output stores on the sync HWDGE queue.
    Transposes/casts for expert e+2 are issued inside expert e's loop so
    tile-pool aliasing never head-of-line-blocks the store stream.
"""

import numpy as np
from contextlib import ExitStack

import bass_rust as _br
import concourse.bass as bass
import concourse.tile as tile
from concourse import mybir
from concourse.bass_utils import run_bass_kernel_spmd

E, CAP, D, H = 32, 4096, 256, 1024
N_CORES = 8
E_PER = E // N_CORES  # 4 experts per core
P = 128
F32 = mybir.dt.float32
F32R = mybir.dt.float32r
BF16 = mybir.dt.bfloat16

KD = D // P        # 2 k-tiles in MM1's contraction
KH = H // P        # 8 k-tiles in MM2's contraction
NC_CHUNK = 512     # capacity chunk processed per MM1/MM2 round == slab size
N_CHUNKS = CAP // NC_CHUNK
H_TILES = H // P
HPACK = 2          # h_tiles packed per PSUM tile / GELU call
NS = NC_CHUNK // P
T_CHUNKS = E_PER * N_CHUNKS  # 32 global (expert, chunk) rounds


def _fix_waits(nc):
    """walrus here accepts only one sync wait per instruction; hoist excess
    waits onto standalone EventSemaphore instructions inserted before the
    offender (same engine => same sequencer order)."""
    for fn in nc.m.functions:
        for bb in fn.blocks:
            new = []
            changed = False
            for inst in bb.instructions:
                si = inst.sync_info
                if si is not None and len(si.on_wait) > 1:
                    waits = list(si.on_wait)
                    for w in waits[:-1]:
                        ev = mybir.InstEventSemaphore(
                            name=nc.get_next_instruction_name()
                        )
                        ev.engine = inst.engine
                        ev.sync_info = _br.SyncInfo(on_wait=[w], on_update=[])
                        nc.register_instruction(ev)
                        new.append(ev)
                    inst.sync_info = _br.SyncInfo(
                        on_wait=waits[-1:], on_update=list(si.on_update)
                    )
                    changed = True
                new.append(inst)
            if changed:
                bb.instructions = new


def _build():
    nc = bass.Bass(trn_type="TRN2", target_bir_lowering=False, debug=False)
    x = nc.dram_tensor("x", [E_PER, CAP, D], F32, kind="ExternalInput").ap()
    w_fc = nc.dram_tensor("w_fc", [E_PER, D, H], F32, kind="ExternalInput").ap()
    w_proj = nc.dram_tensor("w_proj", [E_PER, H, D], F32, kind="ExternalInput").ap()
    out = nc.dram_tensor("out", [E_PER, CAP, D], F32, kind="ExternalOutput").ap()
    # bf16 staging copies of x so the XBar DMA-transpose (2-byte dtype only)
    # can build xT without burning TensorE cycles on identity transposes.
    # One DRAM tensor per (expert, slab): DRAM dependency tracking is
    # tensor-granular, so each transpose starts as soon as its own cast lands.
    # Expert 0 is staged in 512-row slabs (short critical path to the first
    # matmul); the rest use 1024-row slabs (fewer, bigger transposes - each
    # dma_start_transpose costs ~1.5us of fixed sync-engine time).
    SLAB0 = NC_CHUNK                  # 512 rows, 8 slabs for expert 0
    SLAB = 2 * NC_CHUNK               # 1024 rows, 4 slabs for experts 1..3
    def slab_rows(e):
        return SLAB0 if e == 0 else SLAB
    xbf = [
        [
            nc.dram_tensor(f"xbf{e}_{s}", [slab_rows(e), D], BF16).ap()
            for s in range(CAP // slab_rows(e))
        ]
        for e in range(E_PER)
    ]

    with tile.TileContext(nc) as tc, ExitStack() as ctx:
        # 2 experts' worth of xT slabs in flight; ring aliasing gates the
        # transposes of expert e+2 on MM1 of expert e having consumed the slab.
        # xT slab pools: expert 0 has its own (512-wide tiles); experts 1..3
        # share an 8-deep ring of 1024-wide tiles (2 experts in flight; ring
        # aliasing gates expert e+2's transpose on expert e's MM1 reads).
        xtp0 = ctx.enter_context(tc.tile_pool(name="xtp0", bufs=8))
        xtp = ctx.enter_context(tc.tile_pool(name="xtp", bufs=8))
        wload = ctx.enter_context(tc.tile_pool(name="wload", bufs=2))
        wfc_p = ctx.enter_context(tc.tile_pool(name="wfc", bufs=2))
        wproj_p = ctx.enter_context(tc.tile_pool(name="wproj", bufs=2))
        ht_p = ctx.enter_context(tc.tile_pool(name="ht", bufs=8))
        out_p = ctx.enter_context(tc.tile_pool(name="outp", bufs=3))
        ps_h = ctx.enter_context(tc.tile_pool(name="ps_h", bufs=2, space="PSUM"))
        ps_o = ctx.enter_context(tc.tile_pool(name="ps_o", bufs=4, space="PSUM"))

        def load_weights(e):
            # raw f32 loads on the Act HWDGE queue (kept clear of the cast
            # and transpose streams), cast to bf16 on the idle DVE.
            wfc_raw = wload.tile([P, KD, H], F32, tag="wl")
            nc.scalar.dma_start(wfc_raw[:], w_fc[e].rearrange("(k p) h -> p k h", p=P))
            wfc = wfc_p.tile([P, KD, H], BF16, tag="wfc")
            nc.vector.tensor_copy(wfc[:], wfc_raw[:])
            wproj_raw = wload.tile([P, KH, D], F32, tag="wl")
            nc.scalar.dma_start(
                wproj_raw[:], w_proj[e].rearrange("(k p) d -> p k d", p=P)
            )
            wproj = wproj_p.tile([P, KH, D], BF16, tag="wproj")
            nc.vector.tensor_copy(wproj[:], wproj_raw[:])
            return wfc, wproj

        # per-(expert, chunk): (xT tile, column offset) for MM1's moving operand
        xslices = [[None] * N_CHUNKS for _ in range(E_PER)]

        def stage_slab(e, s):
            """cast x[e] slab s to bf16 (gpsimd SWDGE, DRAM->DRAM) and
            XBar-transpose the whole slab with ONE dma_start_transpose:
            [rows, 256] -> [128, KD, rows] (the 3D output folds the d-tile
            index into the partition dim, yielding both k-tiles at once)."""
            rows = slab_rows(e)
            rs = slice(s * rows, (s + 1) * rows)
            nc.gpsimd.dma_start(xbf[e][s][:], x[e][rs])
            pool, tag = (xtp0, "xt0") if e == 0 else (xtp, "xt")
            xt = pool.tile([P, KD, rows], BF16, tag=tag, name=f"xt{e}_{s}")
            nc.sync.dma_start_transpose(xt[:], xbf[e][s][:])
            for c in range(rows // NC_CHUNK):
                xslices[e][s * (rows // NC_CHUNK) + c] = (xt, c * NC_CHUNK)

        # ---- prologue: e0's weights first, then per-slab cast->transpose
        # chains for experts 0 and 1 in consumption order. Experts 2 and 3
        # are staged inside the main loop.
        w = [None] * E_PER
        w[0] = load_weights(0)
        for e in range(min(2, E_PER)):
            for s in range(CAP // slab_rows(e)):
                stage_slab(e, s)

        # pending MM2 work: (e, nci, ht_tiles) of the previous chunk round
        pend = None

        def run_mm2(p_e, p_nci, p_ht, last):
            wproj_t = w[p_e][1]
            psos = [
                ps_o.tile([P, 2 * D], F32, tag="pso",
                          name=f"pso{p_e}_{p_nci}_{i}")
                for i in range(NS)
            ]
            ob = out_p.tile([P, NS, D], F32, tag="ob")
            order = (
                [(s, k) for s in range(NS) for k in range(KH)]
                if last else
                [(s, k) for k in range(KH) for s in range(NS)]
            )
            for s, k in order:
                nc.tensor.matmul(
                    psos[s][:, :D],
                    p_ht[k // HPACK][:, k % HPACK, s * P:(s + 1) * P],
                    wproj_t[:, k, :],
                    start=(k == 0),
                    stop=(k == KH - 1),
                )
                if last and k == KH - 1:
                    # final round: per-subtile eviction+store so the output
                    # tail overlaps the last matmuls
                    nc.vector.tensor_copy(ob[:, s, :], psos[s][:, :D])
                    nc.sync.dma_start(
                        out[p_e, p_nci * NC_CHUNK + s * P:
                            p_nci * NC_CHUNK + (s + 1) * P, :],
                        ob[:, s, :],
                    )
            if not last:
                for s, pso in enumerate(psos):
                    nc.vector.tensor_copy(ob[:, s, :], pso[:, :D])
                csl = slice(p_nci * NC_CHUNK, (p_nci + 1) * NC_CHUNK)
                nc.sync.dma_start(
                    out[p_e, csl, :].rearrange("(s p) d -> p s d", p=P), ob[:]
                )

        for t in range(T_CHUNKS + 1):
            if t < T_CHUNKS:
                e, nci = divmod(t, N_CHUNKS)
                if nci == 0 and e + 1 < E_PER:
                    w[e + 1] = load_weights(e + 1)
                wfc_t = w[e][0]
                # ---- MM1 -> GELU for chunk t ----
                # MM1 accumulates HPACK h_tiles into one 2-bank PSUM tile so
                # GELU evicts in wide ACTIVATE calls; hT is written bf16 so
                # MM2's per-matmul weight loads run at 2-byte FWL speed.
                xt_t, xoff = xslices[e][nci]
                ht_tiles = []
                for hp in range(H_TILES // HPACK):
                    psh = ps_h.tile([P, HPACK, NC_CHUNK], F32, tag="psh")
                    for j in range(HPACK):
                        hi = hp * HPACK + j
                        for k in range(KD):
                            nc.tensor.matmul(
                                psh[:, j, :],
                                wfc_t[:, k, hi * P:(hi + 1) * P],
                                xt_t[:, k, xoff:xoff + NC_CHUNK],
                                start=(k == 0),
                                stop=(k == KD - 1),
                            )
                    ht = ht_p.tile([P, HPACK, NC_CHUNK], BF16, tag="ht")
                    nc.scalar.activation(
                        ht[:], psh[:], mybir.ActivationFunctionType.Gelu
                    )
                    ht_tiles.append(ht)
                # stage expert e+2's slab for this chunk position (keeps the
                # sync-queue transpose stream interleaved with output stores)
                if e + 2 < E_PER and nci % 2 == 0:
                    stage_slab(e + 2, nci // 2)
            # ---- MM2 for the previous chunk round (one-chunk delay: its
            # GELUs completed during this round's MM1, so the PE never
            # waits on the ACT engine) ----
            if pend is not None:
                p_e, p_nci, p_ht = pend
                run_mm2(p_e, p_nci, p_ht, last=(t == T_CHUNKS))
            pend = (e, nci, ht_tiles) if t < T_CHUNKS else None

    _fix_waits(nc)
    return nc


_CACHE = {}


def _get_nc():
    if "nc" not in _CACHE:
        _CACHE["nc"] = _build()
    return _CACHE["nc"]


def kernel(x, w_fc, w_proj, trace=False):
    assert x.shape == (E, CAP, D) and w_fc.shape == (E, D, H)
    assert w_proj.shape == (E, H, D)
    nc = _get_nc()
    x = np.ascontiguousarray(x, dtype=np.float32)
    w_fc = np.ascontiguousarray(w_fc, dtype=np.float32)
    w_proj = np.ascontiguousarray(w_proj, dtype=np.float32)
    in_maps = [
        {
            "x": x[i * E_PER:(i + 1) * E_PER],
            "w_fc": w_fc[i * E_PER:(i + 1) * E_PER],
            "w_proj": w_proj[i * E_PER:(i + 1) * E_PER],
        }
        for i in range(N_CORES)
    ]
    res = run_bass_kernel_spmd(nc, in_maps, list(range(N_CORES)), trace=trace)
    out = np.concatenate([r["out"] for r in res.results], axis=0)
    if trace:
        kernel.last_results = res
    return out
